# revision 3
# baseline (speedup 1.0000x reference)
# Self-contained Trainium2 Bass kernel for the deformable-conv problem.
# kernel(**inputs) takes FULL unsharded inputs, shards batch across 8 cores,
# runs one Bass program SPMD, and reassembles the full output.
#
# STATUS (end of session): HW-bisected to one remaining defect.  VERIFIED ON
# HW: the full prep (image/quad-table build, index+weight pipeline via the
# SBUF-bounce reorder -- plain DRAM->DRAM DMACopy crashes the exec unit and
# must be avoided), all 18-per-image transpose dma_gathers, and the WM
# log-doubling replication (t4_sim.py hw with SKIP_TT/SKIP_MM).  STILL
# CRASHING: enabling the per-slab DVE tensor_tensor multiply (fp16 [128,512]
# contiguous operands) alongside the gathers triggers
# NRT_EXEC_UNIT_UNRECOVERABLE; prime suspect is the documented DVE-perf-mode
# vs GpSimd/SWDGE SBUF port interaction while descriptor generation for the
# gathers is in flight.  Next step: force the TT away from packed modes (f32
# operands or odd alignment) or fence TT against in-flight gather desc-gen.
# The numpy decomposition itself validates at 3.9e-4 rel err (work/t3b.py).
import os

import numpy as np

import concourse.bacc as bacc
import concourse.bass as bass
import concourse.mybir as mybir
import concourse.tile as tile
from concourse.tile import add_dep_helper
from concourse.bass_utils import run_bass_kernel_spmd

f32 = mybir.dt.float32
f16 = mybir.dt.float16
i16 = mybir.dt.int16
i32 = mybir.dt.int32
OP = mybir.AluOpType
P = 128
N9 = 9
C = 64
F = 64
H = 128
W = 128
NPIX = H * W
PXB = 512
DI = PXB // P
NCALLS = NPIX // PXB
CALL = PXB * N9
SPC = CALL // 16
HCH = 64
NCH = H // HCH
MCH = HCH * N9
MFREE = H * N9
XCOL_RANK = H + 1
CORNER_RANK = H + 2
NRANKS = H + 3
NSLOTS = NRANKS * 128
GIDX = PXB
GSP = GIDX // 16
NCORES = 8


def _build_kernel(tc, outs, ins):
    nc = tc.nc
    x_in, xoff, kdup, mi_in, mr_in, jp1_in = ins
    out64 = outs

    with tc.tile_pool(name="dram", bufs=1, space="DRAM") as dpool:
        qt_d = dpool.tile([NSLOTS * 256], f16)
        w4_d = dpool.tile([NCALLS * 4 * CALL], f16)
        idx_d = dpool.tile([NPIX * N9], i16)
        w4f_d = dpool.tile([P * 4 * MFREE], f16)
        idxf_d = dpool.tile([P * MFREE], i16)
        qtt, qto = qt_d[:].tensor, qt_d[:].offset

        def S(name, shape, dtype):
            return nc.alloc_sbuf_tensor("sb_" + name, shape, dtype).ap()

        idxw = S("idxw", [P, SPC * NCALLS], i16)
        G_ring = [S(f"Gr{k}", [P, 2, PXB], f16) for k in range(8)]
        S_ring = [S(f"Sr{k}", [P, 2, PXB], f16) for k in range(8)]
        WM_ring = [S(f"WMr{k}", [P, 2, CALL], f16) for k in range(2)]
        ps_ring = [nc.alloc_psum_tensor(f"psr{k}", [F, PXB], f32).ap()
                   for k in range(4)]
        ob_ring = [S(f"obr{k}", [F, PXB], f32) for k in range(3)]
        kd = S("kd", [P, N9 * F], f16)
        jp1 = S("jp1", [P, 1], f32)
        w4h = S("w4h", [P, 4 * MFREE], f16)
        idxh = S("idxh", [P, MFREE], i16)
        kdsrc = bass.AP(kdup[:].tensor, kdup[:].offset,
                        [[F, P], [P * F, N9], [1, F]])
        nc.sync.dma_start(out=kd[:], in_=kdsrc)
        nc.sync.dma_start(out=jp1[:], in_=jp1_in[:])

        # ---- Stage A: fp16 image in SBUF + quad table in DRAM ----
        xh = S("xh", [H, W * C], f16)
        XCH = W * C // 4
        xf = S("xf", [H, XCH], f32)
        for ch in range(4):
            xsrc = bass.AP(x_in[:].tensor, x_in[:].offset + ch * XCH,
                           [[W * C, H], [1, XCH]])
            nc.sync.dma_start(out=xf[:], in_=xsrc)
            nc.scalar.copy(out=xh[:, ch * XCH:(ch + 1) * XCH], in_=xf[:])
        zt = S("zt", [P, 1024], f16)
        nc.vector.memset(zt[:], 0.0)
        ztotal = NSLOTS * 256
        CHZ = P * 1024
        nz = (ztotal + CHZ - 1) // CHZ
        for z in range(nz):
            lo = z * CHZ
            rows = min(CHZ, ztotal - lo) // 1024
            zdst = bass.AP(qtt, qto + lo, [[1024, rows], [1, 1024]])
            nc.sync.dma_start(out=zdst, in_=zt[0:rows, :])
        xpitch = xh[:].ap[0][0]
        xbase = xh[:].offset
        for dy in range(2):
            for dx in range(2):
                qi = dy * 2 + dx
                y0, y1 = max(0, 1 - dy), min(H, H - dy)
                x0, x1 = max(0, 1 - dx), min(W - 1, W - dx)
                ny, nx = y1 - y0 + 1, x1 - x0 + 1
                dst = bass.AP(qtt, qto + (y0 * 128 + x0) * 256 + qi * C,
                              [[128 * 256, ny], [256, nx], [1, C]])
                src = bass.AP(xh[:].tensor,
                              xbase + (y0 + dy - 1) * xpitch + (x0 + dx - 1) * C,
                              [[xpitch, ny], [C, nx], [1, C]])
                nc.sync.dma_start(out=dst, in_=src)
            yl0, yl1 = max(0, 1 - dy), min(min(H, 127), H - dy)
            nyl = yl1 - yl0 + 1
            if nyl > 0:
                dstx = bass.AP(qtt,
                               qto + (XCOL_RANK * 128 + yl0) * 256 + (dy * 2) * C,
                               [[256, nyl], [1, C]])
                srcx = bass.AP(xh[:].tensor,
                               xbase + (yl0 + dy - 1) * xpitch + (W - 1) * C,
                               [[xpitch, nyl], [1, C]])
                nc.sync.dma_start(out=dstx, in_=srcx)
        if H >= 128:
            dstc = bass.AP(qtt, qto + CORNER_RANK * 128 * 256, [[1, 1], [1, C]])
            srcc = bass.AP(xh[:].tensor, xbase + 127 * xpitch + (W - 1) * C,
                           [[xpitch, 1], [1, C]])
            nc.sync.dma_start(out=dstc, in_=srcc)

        # ---- Stage B: indices & weights ----
        w4h4 = w4h[:].rearrange("p (q n i) -> p q n i", q=4, n=N9, i=H)
        idxh3 = idxh[:].rearrange("p (n i) -> p n i", n=N9, i=H)
        _tc = {}

        def T(name, dtype=f32):
            if name not in _tc:
                _tc[name] = S(name, [P, MCH], dtype)
            return _tc[name]

        for ich in range(NCH):
            ibase = ich * HCH
            xo = T("xo")
            mi = T("mi")
            mr = T("mr")
            xosrc = bass.AP(xoff[:].tensor, xoff[:].offset + ibase * W * N9,
                            [[N9, W], [W * N9, HCH], [1, N9]])
            nc.sync.dma_start(out=xo[:], in_=xosrc)
            nc.sync.dma_start(out=mi[:], in_=mi_in[:, ibase * N9:(ibase + HCH) * N9])
            nc.sync.dma_start(out=mr[:], in_=mr_in[:, ibase * N9:(ibase + HCH) * N9])

            def side(pre, base_is_j):
                rel = T(pre + "rel")
                if base_is_j:
                    nc.vector.tensor_scalar(out=rel[:], in0=xo[:], scalar1=jp1[:, 0:1],
                                            scalar2=None, op0=OP.add)
                else:
                    nc.vector.tensor_tensor(out=rel[:], in0=xo[:], in1=mi[:], op=OP.add)
                nc.vector.tensor_tensor(out=rel[:], in0=rel[:], in1=mr[:], op=OP.add)
                ti = T(pre + "ti", i32)
                tf = T(pre + "tf")
                nc.vector.tensor_copy(out=ti[:], in_=rel[:])
                nc.vector.tensor_copy(out=tf[:], in_=ti[:])
                corr = T(pre + "corr")
                nc.vector.tensor_tensor(out=corr[:], in0=tf[:], in1=rel[:], op=OP.is_gt)
                nc.vector.tensor_tensor(out=tf[:], in0=tf[:], in1=corr[:], op=OP.subtract)
                r0 = tf
                dim = W if base_is_j else H
                c0 = T(pre + "c0")
                nc.vector.tensor_scalar(out=c0[:], in0=r0[:], scalar1=0.0,
                                        scalar2=float(dim + 1), op0=OP.max, op1=OP.min)
                c1 = T(pre + "c1")
                nc.vector.tensor_scalar(out=c1[:], in0=r0[:], scalar1=1.0, scalar2=0.0,
                                        op0=OP.add, op1=OP.max)
                nc.vector.tensor_scalar(out=c1[:], in0=c1[:], scalar1=float(dim + 1),
                                        scalar2=None, op0=OP.min)
                nc.vector.tensor_tensor(out=c1[:], in0=c1[:], in1=rel[:], op=OP.subtract)
                w0 = c1
                nc.vector.tensor_tensor(out=c0[:], in0=rel[:], in1=c0[:], op=OP.subtract)
                w1 = c0
                hi = T(pre + "hi")
                nc.vector.tensor_scalar(out=hi[:], in0=r0[:], scalar1=float(dim + 1),
                                        scalar2=None, op0=OP.is_ge)
                lo = T(pre + "lo")
                nc.vector.tensor_scalar(out=lo[:], in0=r0[:], scalar1=-1.0,
                                        scalar2=None, op0=OP.is_le)
                nc.vector.tensor_tensor(out=hi[:], in0=w0[:], in1=hi[:], op=OP.mult)
                t0 = hi
                nc.vector.tensor_tensor(out=lo[:], in0=w1[:], in1=lo[:], op=OP.mult)
                t1 = lo
                nc.vector.tensor_tensor(out=w0[:], in0=w0[:], in1=t0[:], op=OP.subtract)
                nc.vector.tensor_tensor(out=w0[:], in0=w0[:], in1=t1[:], op=OP.add)
                nc.vector.tensor_tensor(out=w1[:], in0=w1[:], in1=t1[:], op=OP.subtract)
                nc.vector.tensor_tensor(out=w1[:], in0=w1[:], in1=t0[:], op=OP.add)
                nc.vector.tensor_scalar(out=corr[:], in0=r0[:], scalar1=0.0,
                                        scalar2=float(dim), op0=OP.max, op1=OP.min)
                return w0, w1, corr

            B0, B1, bx = side("x", True)
            A0, A1, by = side("y", False)

            prod = T("prod")
            for qi, (ay, bw) in enumerate(((A0, B0), (A0, B1), (A1, B0), (A1, B1))):
                nc.vector.tensor_tensor(out=prod[:], in0=ay[:], in1=bw[:], op=OP.mult)
                nc.vector.tensor_copy(
                    out=w4h4[:, qi, :, ibase:ibase + HCH],
                    in_=prod[:].rearrange("p (i n) -> p n i", i=HCH, n=N9))

            m128 = T("xrel")
            nc.vector.tensor_scalar(out=m128[:], in0=bx[:], scalar1=float(W),
                                    scalar2=None, op0=OP.is_ge)
            idxa = T("yrel")
            nc.vector.tensor_scalar(out=idxa[:], in0=by[:], scalar1=128.0,
                                    scalar2=None, op0=OP.mult)
            nc.vector.tensor_tensor(out=idxa[:], in0=idxa[:], in1=bx[:], op=OP.add)
            idxb = T("xhi")
            nc.vector.tensor_scalar(out=idxb[:], in0=by[:],
                                    scalar1=float(XCOL_RANK * 128),
                                    scalar2=None, op0=OP.add)
            nc.vector.tensor_tensor(out=idxb[:], in0=idxb[:], in1=idxa[:], op=OP.subtract)
            nc.vector.tensor_tensor(out=idxb[:], in0=idxb[:], in1=m128[:], op=OP.mult)
            nc.vector.tensor_tensor(out=idxa[:], in0=idxa[:], in1=idxb[:], op=OP.add)
            nc.vector.tensor_copy(
                out=idxh3[:, :, ibase:ibase + HCH],
                in_=idxa[:].rearrange("p (i n) -> p n i", i=HCH, n=N9))

        # flat dumps then DRAM->DRAM reorder into [c,q,n,j,di] / [c,n,j,di]
        w4fdst = bass.AP(w4f_d[:].tensor, w4f_d[:].offset,
                         [[4 * MFREE, P], [1, 4 * MFREE]])
        nc.sync.dma_start(out=w4fdst, in_=w4h[:])
        idxfdst = bass.AP(idxf_d[:].tensor, idxf_d[:].offset,
                          [[MFREE, P], [1, MFREE]])
        nc.sync.dma_start(out=idxfdst, in_=idxh[:])
        bw = S("bw", [P, H], f16)
        bi = S("bi", [P, H], i16)
        for q in range(4):
            for n in range(N9):
                hsrc = bass.AP(w4f_d[:].tensor,
                               w4f_d[:].offset + (q * N9 + n) * H,
                               [[4 * MFREE, P], [1, H]])
                nc.sync.dma_start(out=bw[:], in_=hsrc)
                w4dst = bass.AP(w4_d[:].tensor,
                                w4_d[:].offset + q * CALL + n * PXB,
                                [[DI, P], [4 * CALL, NCALLS], [1, DI]])
                bsrc = bass.AP(bw[:].tensor, bw[:].offset,
                               [[bw[:].ap[0][0], P], [DI, NCALLS], [1, DI]])
                nc.sync.dma_start(out=w4dst, in_=bsrc)
        for n in range(N9):
            hsrc = bass.AP(idxf_d[:].tensor, idxf_d[:].offset + n * H,
                           [[MFREE, P], [1, H]])
            nc.sync.dma_start(out=bi[:], in_=hsrc)
            idst = bass.AP(idx_d[:].tensor, idx_d[:].offset + n * PXB,
                           [[DI, P], [CALL, NCALLS], [1, DI]])
            bsrc = bass.AP(bi[:].tensor, bi[:].offset,
                           [[bi[:].ap[0][0], P], [DI, NCALLS], [1, DI]])
            nc.sync.dma_start(out=idst, in_=bsrc)
        ipitch = idxw[:].ap[0][0]
        for rep in range(8):
            iwdst = bass.AP(idxw[:].tensor, idxw[:].offset + rep * 16 * ipitch,
                            [[ipitch, 16], [SPC, NCALLS], [1, SPC]])
            iwsrc = bass.AP(idx_d[:].tensor, idx_d[:].offset,
                            [[1, 16], [CALL, NCALLS], [16, SPC]])
            nc.sync.dma_start(out=iwdst, in_=iwsrc)

        # ---- main loop ----
        qtv = qt_d[:].rearrange("(s e) -> s e", s=NSLOTS, e=256)
        prev_gathers = []
        ob_insts = [None, None, None, None]
        for cc in range(NCALLS):
            WM = WM_ring[cc % 2]
            wmp = WM[:].ap[0][0]
            wm_insts = []
            for dxh in range(2):
                # seed partition row dxh*64 with [w(dy=0,dx), w(dy=1,dx)]
                wmdst = bass.AP(WM[:].tensor, WM[:].offset + dxh * C * wmp,
                                [[wmp, 1], [CALL, 2], [1, CALL]])
                wmsrc = bass.AP(w4_d[:].tensor,
                                w4_d[:].offset + (cc * 4 + dxh) * CALL,
                                [[wmp, 1], [2 * CALL, 2], [1, CALL]])
                wmi = nc.scalar.dma_start(out=wmdst, in_=wmsrc)
                for g in prev_gathers:
                    add_dep_helper(wmi.ins, g.ins, True, "xbar serialization")
                wm_insts.append(wmi)
            for k in range(6):
                sz = 1 << k
                for dxh in range(2):
                    base = dxh * C
                    ddst = bass.AP(WM[:].tensor, WM[:].offset + (base + sz) * wmp,
                                   [[wmp, sz], [1, 2 * CALL]])
                    dsrc = bass.AP(WM[:].tensor, WM[:].offset + base * wmp,
                                   [[wmp, sz], [1, 2 * CALL]])
                    wmi = nc.scalar.dma_start(out=ddst, in_=dsrc)
                    for g in prev_gathers:
                        add_dep_helper(wmi.ins, g.ins, True, "xbar serialization")
                    wm_insts.append(wmi)
            prev_gathers = []
            ps = ps_ring[cc % 4]
            for n in range(N9):
                G = G_ring[(cc * N9 + n) % 8]
                gi = nc.gpsimd.dma_gather(
                    G[:], qtv,
                    idxw[:, cc * SPC + n * GSP: cc * SPC + (n + 1) * GSP],
                    num_idxs=GIDX, num_idxs_reg=GIDX, elem_size=256,
                    transpose=True, queue_num=(cc * N9 + n) % 4)
                for w in wm_insts:
                    add_dep_helper(gi.ins, w.ins, True, "gather after WM load")
                prev_gathers.append(gi)
                Sg = S_ring[(cc * N9 + n) % 8]
                for dy in range(2):
                    nc.gpsimd.tensor_tensor(
                        out=Sg[:, dy, :], in0=G[:, dy, :],
                        in1=WM[:, dy, n * PXB:(n + 1) * PXB], op=OP.mult)
                lhsT = kd[:, n * F:(n + 1) * F]
                for dy in range(2):
                    mmi = nc.tensor.matmul(
                        ps[:], lhsT, Sg[:, dy, :],
                        start=(n == 0 and dy == 0),
                        stop=(n == N9 - 1 and dy == 1))
                    if n == 0 and dy == 0 and ob_insts[cc % 4] is not None:
                        add_dep_helper(mmi.ins, ob_insts[cc % 4].ins, True, "psum reuse")
            ob = ob_ring[cc % 3]
            obi = nc.scalar.copy(out=ob[:], in_=ps[:])
            ob_insts[cc % 4] = obi
            nc.sync.dma_start(out=out64[:, cc * PXB:(cc + 1) * PXB], in_=ob[:])


def _make_consts(kernel_np):
    k9 = kernel_np.reshape(N9, C, F)
    kdup = np.concatenate([k9, k9], axis=1).astype(np.float16)
    ii = np.repeat(np.arange(H, dtype=np.float32) + 1.0, N9)
    R = np.tile(np.arange(-1, 2, dtype=np.float32), 3)
    rr = np.tile(R, H)
    mi = np.broadcast_to(ii, (P, H * N9)).copy()
    mr = np.broadcast_to(rr, (P, H * N9)).copy()
    jp1 = (np.arange(P, dtype=np.float32) + 1.0).reshape(P, 1)
    return kdup, mi, mr, jp1


_CACHE = {}


def _get_nc():
    if "nc" in _CACHE:
        return _CACHE["nc"]
    nc = bacc.Bacc("TRN2", target_bir_lowering=False, debug=False,
                   num_swdge_queues=4, detect_race_conditions=False)
    x = nc.dram_tensor("x", [H, W, C], f32, kind="ExternalInput")
    xo = nc.dram_tensor("xoff", [H, W, N9], f32, kind="ExternalInput")
    kdin = nc.dram_tensor("kdin", [N9, P, F], f16, kind="ExternalInput")
    mi = nc.dram_tensor("mi_in", [P, H * N9], f32, kind="ExternalInput")
    mr = nc.dram_tensor("mr_in", [P, H * N9], f32, kind="ExternalInput")
    jp = nc.dram_tensor("jp_in", [P, 1], f32, kind="ExternalInput")
    out = nc.dram_tensor("out64", [F, NPIX], f32, kind="ExternalOutput")
    with tile.TileContext(nc) as tc:
        _build_kernel(tc, out.ap(),
                      (x.ap(), xo.ap(), kdin.ap(), mi.ap(), mr.ap(), jp.ap()))
    nc.compile()
    _CACHE["nc"] = nc
    return nc


def kernel(x_in, y_offset, x_offset, kernel):
    x_in = np.ascontiguousarray(x_in, dtype=np.float32)
    x_offset = np.ascontiguousarray(x_offset, dtype=np.float32)
    kdup, mi, mr, jp1 = _make_consts(np.asarray(kernel, dtype=np.float32))
    nc = _get_nc()
    in_maps = []
    for b in range(NCORES):
        in_maps.append({
            "x": x_in[b],
            "xoff": x_offset[b],
            "kdin": kdup,
            "mi_in": mi,
            "mr_in": mr,
            "jp_in": jp1,
        })
    trace = bool(os.environ.get("DEFCONV_TRACE"))
    kw = {}
    if trace:
        kw["trace"] = True
        td = os.environ.get("DEFCONV_TRACE_DIR")
        if td:
            os.makedirs(td, exist_ok=True)
            kw["tmpdir"] = td
    res = run_bass_kernel_spmd(nc, in_maps, core_ids=list(range(NCORES)), **kw)
    global LAST_EXEC_NS, LAST_TRACE
    LAST_EXEC_NS = getattr(res, "exec_time_ns", None) or -1
    it = getattr(res, "instructions_and_trace", None)
    LAST_TRACE = it[1] if it else None
    out = np.empty((NCORES, H, W, F), np.float32)
    for b in range(NCORES):
        o64 = res.results[b]["out64"]            # [F, NPIX] in stream order
        # stream position (cc, j, di) -> pixel (i=cc*DI+di, j)
        o = o64.reshape(F, NCALLS, W, DI)        # [f, cc, j, di]
        o = o.transpose(1, 3, 2, 0)              # [cc, di, j, f]
        out[b] = o.reshape(H, W, F)
    return out

